# revision 2
# baseline (speedup 1.0000x reference)
"""Trainium2 Bass kernel for nn_ChemModel (DMPNN-style message-passing GNN), v2.

Self-contained: call kernel(**inputs) with the full (unsharded) inputs from
setup_inputs(); returns the full [N_GRAPHS, 1] float32 output.

Strategy (8 NeuronCores, SPMD — one program, per-core data):
  * Nodes/slots sharded in contiguous dst ranges of N/8 = 12500 per core.
    Persistent transposed state hA_T [128h x 12544] f32 lives in SBUF.
  * h0 (init and the e>=N stream of the final aggregation) is computed
    directly from host-prepared [x[src] | edge_attr] inputs — no gathers.
  * The per-iteration message table [8*12544, 128] bf16 is produced by ONE
    AllGather per iteration; int16 gather indices address one of 4
    core-pair windows (25088 rows) of it.
  * Per-core edges are partitioned by src core-pair into 4 sublists, each
    sorted by local dst and packed into per-dst-block buckets whose
    capacities (64-slot granularity) are shared across cores, keeping the
    chunk->block mapping SPMD-uniform. Messages are fetched with ONE
    dma_gather pass straight from the quarter table (no staging buffer /
    re-gather). Scatter-add over dst uses one-hot matmuls accumulated per
    128-slot dst block in PSUM (<=3 segments per 128-edge chunk).
  * Final pass: hfin quarter-tables (final h for edge ids < N) gathered the
    same way; the e>=N stream is computed inline; node embeddings,
    512-wide one-hot graph pooling, AllGather + baked-offset assembly of
    the pooled [G,128], and a small replicated FFN.
"""
import math
import numpy as np

import concourse.bass as bass
from concourse import bacc
import concourse.mybir as mybir
import concourse.tile as tile
from concourse.bass_utils import run_bass_kernel_spmd
from concourse import library_config

P = 128
NCORES = 8
GIDX = 2048                # indices per dma_gather instruction
GCH = GIDX // P            # chunks per gather instruction (16)
BUCK = 64                  # bucket granularity (slots)
F32 = mybir.dt.float32
BF16 = mybir.dt.bfloat16
FP16 = mybir.dt.float16
I16 = mybir.dt.int16
MSG_DT = BF16
MAXSEG = 3                 # chunk spans <= 128/BUCK + 1 buckets


def _relu():
    return mybir.ActivationFunctionType.Relu


def _wrap_idx16(flat):
    """[n] int array (n % 16 == 0) -> [128, n//16] int16 wrapped layout."""
    n = flat.shape[0]
    assert n % 16 == 0
    w = flat.reshape(n // 16, 16).T.astype(np.int16)
    return np.tile(w, (8, 1))


class _Plan:
    pass


class _SharedSub:
    """Shared (cross-core) structure of one gather+scatter sublist."""
    pass


def _build_shared_sublist(per_core, nblocks, gidx=GIDX):
    """per_core: list of (rows, dloc) per core, dloc sorted ascending.

    Returns (_SharedSub, per-core list of dicts with idx [npad] and
    dlocf [128, nch] float arrays).
    """
    ncores = len(per_core)
    counts = np.zeros((ncores, nblocks), np.int64)
    for k, (rows, dloc) in enumerate(per_core):
        if len(dloc):
            counts[k] = np.bincount(dloc // P, minlength=nblocks)
    caps = ((counts.max(axis=0) + BUCK - 1) // BUCK) * BUCK  # slots per block
    starts = np.concatenate([[0], np.cumsum(caps)])
    total = int(starts[-1])
    npad = max(gidx, ((total + gidx - 1) // gidx) * gidx)
    nch = npad // P

    # chunk -> ordered list of blocks intersecting it
    blists = []
    touches = {}
    bi = 0  # current block cursor
    nzblocks = np.where(caps > 0)[0]
    # build slot->block map (coarse, via bucket boundaries)
    blk_of_slot = np.full(npad, -1, np.int64)
    for b in nzblocks:
        blk_of_slot[starts[b]:starts[b + 1]] = b
    for c in range(nch):
        blks = np.unique(blk_of_slot[c * P:(c + 1) * P])
        blks = [int(b) for b in blks if b >= 0]
        assert len(blks) <= MAXSEG
        blists.append(blks)
        for s, b in enumerate(blks):
            touches.setdefault(b, []).append((c, s))
    first, last = {}, {}
    for b, lst in touches.items():
        first[lst[0]] = b
        last[lst[-1]] = b

    sh = _SharedSub()
    sh.caps, sh.starts, sh.total = caps, starts, total
    sh.npad, sh.nch, sh.ninstr = npad, nch, npad // gidx
    sh.blists, sh.first, sh.last = blists, first, last

    # per-slot dloc offset base: for slot in block b within chunk c,
    # dlocf = (dloc - b*128) + 128 * blists[c].index(b)
    seg_of_slot = np.full(npad, -1, np.int64)
    for c in range(nch):
        for s, b in enumerate(blists[c]):
            sl = np.arange(c * P, (c + 1) * P)
            m = blk_of_slot[sl] == b
            seg_of_slot[sl[m]] = s

    percore = []
    for k, (rows, dloc) in enumerate(per_core):
        idx = np.zeros(npad, np.int64)
        dv = np.full(npad, -1.0, np.float32)
        off = 0
        pos = np.zeros(nblocks, np.int64)
        # place each edge at starts[block] + running offset
        blk = dloc // P if len(dloc) else np.zeros(0, np.int64)
        # edges sorted by dloc -> grouped by block already
        for b in nzblocks:
            m = blk == b
            nb_ = int(m.sum())
            if nb_ == 0:
                continue
            s0 = starts[b]
            idx[s0:s0 + nb_] = rows[m]
            dv[s0:s0 + nb_] = (dloc[m] - b * P)
        dv_seg = dv.copy()
        real = dv >= 0
        dv_seg[real] += 128.0 * seg_of_slot[real]
        assert (seg_of_slot[real] >= 0).all()
        percore.append({
            "idx": idx,
            "dlocf": dv_seg.reshape(nch, P).T.astype(np.float16),
        })
    return sh, percore


def _host_prep(x, edge_index, edge_attr, batch, depth, G):
    N, E = x.shape[0], edge_index.shape[1]
    H = 128
    src = edge_index[0].astype(np.int64)
    dst = edge_index[1].astype(np.int64)
    batch = batch.astype(np.int64)
    x = np.asarray(x, np.float32)
    edge_attr = np.asarray(edge_attr, np.float32)

    assert N % NCORES == 0
    NSH = N // NCORES                      # 12500
    NB = math.ceil(NSH / P)                # 98
    SHP = NB * P                           # 12544
    RNG = 2 * SHP                          # gather window rows (25088)
    assert RNG <= 32512

    def row_win(v):
        """window id and row-within-window of node/slot v."""
        c = v // NSH
        l = v % NSH
        return c // 2, (c % 2) * SHP + l

    q_src, row_src = row_win(src)

    plan = _Plan()
    plan.N, plan.E, plan.H, plan.G = N, E, H, G
    plan.NSH, plan.NB, plan.SHP, plan.RNG = NSH, NB, SHP, RNG
    plan.depth = int(depth)
    plan.GW = min(512, G)

    # per-core edge sets sorted by local dst
    core_of = dst // NSH
    edges = []
    for k in range(NCORES):
        eidx = np.where(core_of == k)[0]
        dloc = dst[eidx] - k * NSH
        order = np.argsort(dloc, kind="stable")
        edges.append((eidx[order], dloc[order]))

    # main loop sublists (by src quarter)
    plan.shL = []
    pcL = [[] for _ in range(NCORES)]
    for q in range(4):
        per_core = []
        for k in range(NCORES):
            eidx, dloc = edges[k]
            m = q_src[eidx] == q
            per_core.append((row_src[eidx[m]], dloc[m]))
        sh, pc = _build_shared_sublist(per_core, NB)
        plan.shL.append(sh)
        for k in range(NCORES):
            pcL[k].append(pc[k])

    # final phase stream-0 (e < N, table row by window of e)
    plan.shF = []
    pcF = [[] for _ in range(NCORES)]
    qe_all, rowe_all = row_win(np.arange(N))
    for q in range(4):
        per_core = []
        for k in range(NCORES):
            eidx, dloc = edges[k]
            m = (eidx < N) & (qe_all[np.minimum(eidx, N - 1)] == q)
            per_core.append((rowe_all[eidx[m]], dloc[m]))
        sh, pc = _build_shared_sublist(per_core, NB)
        plan.shF.append(sh)
        for k in range(NCORES):
            pcF[k].append(pc[k])

    # final phase stream-1 (e >= N, computed h0)
    per_core1 = []
    for k in range(NCORES):
        eidx, dloc = edges[k]
        m = eidx >= N
        per_core1.append((np.zeros(int(m.sum()), np.int64), dloc[m]))
    plan.sh1, pc1 = _build_shared_sublist(per_core1, NB)

    cores = []
    for k in range(NCORES):
        info = {}
        info["subL"] = pcL[k]
        info["subF"] = pcF[k]
        info["sub1"] = pc1[k]
        eidx, dloc = edges[k]
        m1 = eidx >= N
        e1 = eidx[m1]
        d1 = dloc[m1]
        # place stream-1 inputs at their slots
        xea1 = np.zeros((7, plan.sh1.npad), np.float32)
        blk = d1 // P
        for b in np.where(plan.sh1.caps > 0)[0]:
            mm = blk == b
            nb_ = int(mm.sum())
            if nb_ == 0:
                continue
            s0 = plan.sh1.starts[b]
            xea1[:4, s0:s0 + nb_] = x[src[e1[mm]]].T
            xea1[4:, s0:s0 + nb_] = edge_attr[e1[mm]].T
        info["xea1"] = np.ascontiguousarray(xea1)

        # init inputs [7, SHP]
        sl = np.arange(k * NSH, (k + 1) * NSH)
        xeai = np.zeros((7, SHP), np.float32)
        xeai[:4, :NSH] = x[src[sl]].T
        xeai[4:, :NSH] = edge_attr[sl].T
        info["xeai"] = np.ascontiguousarray(xeai)

        xs = np.zeros((SHP, 4), np.float32)
        xs[:NSH] = x[k * NSH:(k + 1) * NSH]
        info["xT"] = np.ascontiguousarray(xs.T)
        cores.append(info)

    # pooling bases
    plan.g_bases = []
    for k in range(NCORES):
        gb = int(batch[k * NSH])
        ge = int(batch[(k + 1) * NSH - 1])
        assert ge - gb < plan.GW, f"graph span {ge - gb} >= {plan.GW}"
        plan.g_bases.append(gb)
    for k in range(NCORES):
        info = cores[k]
        bl = np.full((SHP,), -1.0, np.float32)
        bl[:NSH] = batch[k * NSH:(k + 1) * NSH] - plan.g_bases[k]
        info["batchloc"] = np.ascontiguousarray(bl.reshape(NB, P).T)

    plan.cores = cores
    return plan


# ----------------------------------------------------------------------------
# device kernel
# ----------------------------------------------------------------------------

def _build(plan, split=True, nreps=1):
    H, NB, SHP = plan.H, plan.NB, plan.SHP
    RNG = plan.RNG
    TBL = NCORES * SHP
    G, GW = plan.G, plan.GW
    depth = plan.depth

    nc = bacc.Bacc(num_devices=NCORES, num_swdge_queues=4)

    def din(name, shape, dt=F32):
        return nc.declare_dram_parameter(name, list(shape), dt, isOutput=False)

    WmT = din("WmT", [H, H])
    WieXT = din("WieXT", [7, H])
    WaxT = din("WaxT", [4, H])
    WahT = din("WahT", [H, H])
    W1T = din("W1T", [H, 4 * H])
    W2T = din("W2T", [4 * H, H])
    WlastT = din("WlastT", [H, 1])
    b1r = din("b1r", [H, 4])
    b2r = din("b2r", [H, 1])
    blast = din("blast", [1, 1])
    iotaW_in = din("iotaW", [P, MAXSEG * P], FP16)
    iotaG_in = din("iotaG", [P, GW])
    ident_in = din("ident", [P, P])
    xT_in = din("xT", [4, SHP])
    xeai_in = din("xeai", [7, SHP])
    xea1_in = din("xea1", [7, plan.sh1.npad])
    batchloc_in = din("batchloc", [P, NB])

    nIL = sum(sh.ninstr for sh in plan.shL)
    nIF = sum(sh.ninstr for sh in plan.shF)
    gaL_in = din("gaL", [P, nIL * (GIDX // 16)], I16)
    gaF_in = din("gaF", [P, nIF * (GIDX // 16)], I16)
    dlocL_in = din("dlocL", [P, sum(sh.nch for sh in plan.shL)], FP16)
    dlocF_in = din("dlocF", [P, sum(sh.nch for sh in plan.shF)], FP16)
    dloc1_in = din("dloc1", [P, plan.sh1.nch], FP16)

    out_ext = nc.declare_dram_parameter("out", [G, 1], F32, isOutput=True)

    RG = list(range(NCORES))

    with tile.TileContext(nc) as tc:
        nc.gpsimd.load_library(library_config.mlp)
        with (
            tc.tile_pool(name="cp", bufs=1) as cp,
            tc.tile_pool(name="sb", bufs=3) as sb,
            tc.tile_pool(name="ps", bufs=2, space="PSUM") as ps,
            tc.tile_pool(name="dr", bufs=1, space="DRAM") as dr,
        ):
            def cload(name, src):
                tl = cp.tile([src.shape[0], src.shape[1]], src.dtype, name=name)
                nc.sync.dma_start(out=tl[:], in_=src[:, :])
                return tl

            WmT_s = cload("WmT_s", WmT)
            WieXT_s = cload("WieXT_s", WieXT)
            WaxT_s = cload("WaxT_s", WaxT)
            WahT_s = cload("WahT_s", WahT)
            W1T_s = cload("W1T_s", W1T)
            W2T_f = []
            for f in range(4):
                tl = cp.tile([P, H], F32, name=f"W2T_{f}")
                nc.sync.dma_start(out=tl[:], in_=W2T[f * P:(f + 1) * P, :])
                W2T_f.append(tl)
            WlastT_s = cload("WlastT_s", WlastT)
            b1r_s = cload("b1r_s", b1r)
            b2r_s = cload("b2r_s", b2r)
            blast_s = cload("blast_s", blast)
            iotaW_s = cload("iotaW_s", iotaW_in)
            iotaG_s = cload("iotaG_s", iotaG_in)
            ident_s = cload("ident_s", ident_in)
            batchloc_s = cload("batchloc_s", batchloc_in)
            gaL_s = cload("gaL_s", gaL_in)
            dlocL_s = cload("dlocL_s", dlocL_in)

            hA_T = cp.tile([P, SHP], F32, name="hA_T")

            def allgather(local, table):
                nc.gpsimd.collective_compute(
                    "AllGather", mybir.AluOpType.bypass,
                    replica_groups=[RG], ins=[local[:]], outs=[table[:]])

            def gather(dst_ap, table_ap, idx_sb, inst_col):
                nc.gpsimd.dma_gather(
                    out_ap=dst_ap, in_ap=table_ap,
                    idxs_ap=idx_sb[:, inst_col * (GIDX // 16):
                                   (inst_col + 1) * (GIDX // 16)],
                    num_idxs=GIDX, num_idxs_reg=GIDX,
                    elem_size=H, single_packet=False,
                    queue_num=inst_col % 4)

            def iota3d(n, seg):
                return bass.AP(iotaW_s.tensor,
                               iotaW_s.offset + seg * P,
                               [list(iotaW_s.ap[0]), [0, n], [1, P]])

            rep_ctx = {}
            for _rep in range(nreps):
              mA_loc = [dr.tile([SHP, H], MSG_DT,
                                name=f"mA_loc_{_rep}_{it}")
                        for it in range(depth)]
              mA_tbl = [dr.tile([TBL, H], MSG_DT, name=f"mA_tbl_{_rep}_{it}",
                                addr_space="Shared") for it in range(depth)]
              hf_loc = dr.tile([SHP, H], MSG_DT, name=f"hf_loc_{_rep}")
              hf_tbl = dr.tile([TBL, H], MSG_DT, name=f"hf_tbl_{_rep}",
                               addr_space="Shared")
              gwin_local = dr.tile([P, GW], F32, name=f"gwin_local_{_rep}")
              gwin_all = dr.tile([NCORES * P, GW], F32,
                                 name=f"gwin_all_{_rep}",
                                 addr_space="Shared")
              # ---------------- init hA_0 (computed, transposed) -------------
            ICH = 25                     # blocks per xeai chunk
            for c0 in range(0, NB, ICH):
                cn = min(ICH, NB - c0)
                xc = sb.tile([7, ICH * P], F32, name="xeic", tag="xea",
                             bufs=2)
                nc.sync.dma_start(out=xc[:, :cn * P],
                                  in_=xeai_in[:, c0 * P:(c0 + cn) * P])
                for bi in range(cn):
                    b = c0 + bi
                    ph = ps.tile([P, P], F32, name="ph0", tag="ptmp",
                                 space="PSUM")
                    nc.tensor.matmul(out=ph[:], lhsT=WieXT_s[:],
                                     rhs=xc[:, bi * P:(bi + 1) * P],
                                     start=True, stop=True)
                    nc.scalar.activation(out=hA_T[:, b * P:(b + 1) * P],
                                         in_=ph[:], func=_relu())

            # ---------------- scatter helper -------------------------------
            def scatter_sublist(msgs_of_group, sh, dloc_s, dloc_col0,
                                target_T):
                blists = sh.blists
                first, last = sh.first, sh.last
                open_ps = {}
                ngroups = (sh.nch + GCH - 1) // GCH
                for g in range(ngroups):
                    gt = msgs_of_group(g)
                    c0 = g * GCH
                    cn = min(GCH, sh.nch - c0)
                    dsl = dloc_s[:, dloc_col0 + c0:dloc_col0 + c0 + cn]
                    oh = sb.tile([P, cn, P], MSG_DT, name="oh", tag="oh")
                    nc.vector.tensor_tensor(
                        out=oh[:], in0=dsl.to_broadcast([P, cn, P]),
                        in1=iota3d(cn, 0), op=mybir.AluOpType.is_equal)
                    oh2 = {}
                    for ci in range(c0, c0 + cn):
                        nseg = len(blists[ci])
                        if nseg > 1:
                            t2 = sb.tile([P, nseg - 1, P], MSG_DT,
                                         name="oh2", tag="oh2")
                            dsl1 = dloc_s[:, dloc_col0 + ci:dloc_col0 + ci + 1]
                            nc.vector.tensor_tensor(
                                out=t2[:],
                                in0=dsl1.to_broadcast([P, nseg - 1, P]),
                                in1=iota3d(nseg - 1, 1),
                                op=mybir.AluOpType.is_equal)
                            oh2[ci] = t2
                    for ci in range(c0, c0 + cn):
                        for s, b in enumerate(blists[ci]):
                            key = (ci, s)
                            if key in first:
                                open_ps[b] = ps.tile([P, H], F32, name="pacc",
                                                     tag="pacc", space="PSUM",
                                                     bufs=4)
                            rhs = (oh[:, ci - c0, :] if s == 0
                                   else oh2[ci][:, s - 1, :])
                            isl = key in last
                            nc.tensor.matmul(out=open_ps[b][:],
                                             lhsT=gt[:, ci - c0, :], rhs=rhs,
                                             start=(key in first), stop=isl)
                            if isl:
                                nc.vector.tensor_add(
                                    out=target_T[:, b * P:(b + 1) * P],
                                    in0=target_T[:, b * P:(b + 1) * P],
                                    in1=open_ps.pop(b)[:])
                assert not open_ps

            # ---------------- message-passing iterations ------------------
            for it in range(depth):
                for b in range(NB):
                    pm = ps.tile([P, H], F32, name="pm", tag="ptmp",
                                 space="PSUM")
                    nc.tensor.matmul(out=pm[:],
                                     lhsT=hA_T[:, b * P:(b + 1) * P],
                                     rhs=WmT_s[:], start=True, stop=True)
                    tm = sb.tile([P, H], MSG_DT, name="tm", tag="sml")
                    nc.scalar.activation(out=tm[:], in_=pm[:], func=_relu())
                    nc.sync.dma_start(out=mA_loc[it][b * P:(b + 1) * P, :],
                                      in_=tm[:])
                allgather(mA_loc[it], mA_tbl[it])

                icol = 0
                ccol = 0
                for q in range(4):
                    sh = plan.shL[q]

                    def msgs(g, _q=q, _it=it, _i0=icol):
                        gt = sb.tile([P, GCH, H], MSG_DT, name="msg",
                                     tag="msg")
                        gather(gt[:], mA_tbl[_it][_q * RNG:(_q + 1) * RNG, :],
                               gaL_s, _i0 + g)
                        return gt

                    scatter_sublist(msgs, sh, dlocL_s, ccol, hA_T)
                    icol += sh.ninstr
                    ccol += sh.nch

            # ---------------- final aggregation ---------------------------
            for b in range(NB):
                pt = ps.tile([P, H], F32, name="ptf", tag="ptmp",
                             space="PSUM")
                nc.tensor.transpose(out=pt[:],
                                    in_=hA_T[:, b * P:(b + 1) * P],
                                    identity=ident_s[:])
                tf = sb.tile([P, H], MSG_DT, name="tf", tag="sml")
                nc.vector.tensor_copy(out=tf[:], in_=pt[:])
                nc.sync.dma_start(out=hf_loc[b * P:(b + 1) * P, :],
                                  in_=tf[:])
            allgather(hf_loc, hf_tbl)

            # hA_T's contents are fully exported to hf_loc above; reuse the
            # same SBUF as the node-embedding accumulator.
            nacc_T = hA_T
            nc.vector.memset(nacc_T[:], 0.0)

            gaF_s = cload("gaF_s", gaF_in)
            dlocF_s = cload("dlocF_s", dlocF_in)
            dloc1_s = cload("dloc1_s", dloc1_in)

            def msgs1(g):
                xt = sb.tile([7, GIDX], F32, name="xea1t", tag="xea", bufs=2)
                nc.sync.dma_start(out=xt[:],
                                  in_=xea1_in[:, g * GIDX:(g + 1) * GIDX])
                ot = sb.tile([P, GCH, H], MSG_DT, name="msg", tag="msg")
                for c in range(GCH):
                    ph = ps.tile([P, H], F32, name="ph1", tag="ptmp",
                                 space="PSUM")
                    nc.tensor.matmul(out=ph[:],
                                     lhsT=xt[:, c * P:(c + 1) * P],
                                     rhs=WieXT_s[:], start=True, stop=True)
                    nc.scalar.activation(out=ot[:, c, :], in_=ph[:],
                                         func=_relu())
                return ot

            scatter_sublist(msgs1, plan.sh1, dloc1_s, 0, nacc_T)

            icol = 0
            ccol = 0
            for q in range(4):
                sh = plan.shF[q]

                def msgsF(g, _q=q, _i0=icol):
                    gt = sb.tile([P, GCH, H], MSG_DT, name="msg", tag="msg")
                    gather(gt[:], hf_tbl[_q * RNG:(_q + 1) * RNG, :],
                           gaF_s, _i0 + g)
                    return gt

                scatter_sublist(msgsF, sh, dlocF_s, ccol, nacc_T)
                icol += sh.ninstr
                ccol += sh.nch

            # ---------------- node embedding + pooling ---------------------
            gps = ps.tile([P, GW], F32, name="gps", tag="gps", space="PSUM",
                          bufs=1)
            for b in range(NB):
                pt = ps.tile([P, H], F32, name="ptr", tag="ptmp", space="PSUM")
                nc.tensor.transpose(out=pt[:], in_=nacc_T[:, b * P:(b + 1) * P],
                                    identity=ident_s[:])
                tT = sb.tile([P, H], F32, name="tT", tag="sml")
                nc.vector.tensor_copy(out=tT[:], in_=pt[:])
                xts = sb.tile([4, P], F32, name="xts", tag="xts")
                nc.sync.dma_start(out=xts[:], in_=xT_in[:, b * P:(b + 1) * P])
                p2 = ps.tile([P, H], F32, name="p2f", tag="ptmp", space="PSUM")
                nc.tensor.matmul(out=p2[:], lhsT=tT[:], rhs=WahT_s[:],
                                 start=True, stop=False)
                nc.tensor.matmul(out=p2[:], lhsT=xts[:], rhs=WaxT_s[:],
                                 start=False, stop=True)
                ne2 = sb.tile([P, H], F32, name="ne2", tag="sml")
                nc.scalar.activation(out=ne2[:], in_=p2[:], func=_relu())
                ohg = sb.tile([P, GW], F32, name="ohg", tag="ohg")
                nc.vector.tensor_tensor(
                    out=ohg[:],
                    in0=batchloc_s[:, b:b + 1].to_broadcast([P, GW]),
                    in1=iotaG_s[:], op=mybir.AluOpType.is_equal)
                nc.tensor.matmul(out=gps[:], lhsT=ne2[:], rhs=ohg[:],
                                 start=(b == 0), stop=(b == NB - 1))

            tgw = sb.tile([P, GW], F32, name="tgw", tag="ohg")
            nc.vector.tensor_copy(out=tgw[:], in_=gps[:])
            nc.sync.dma_start(out=gwin_local[:, :], in_=tgw[:])
            allgather(gwin_local, gwin_all)
            gfull = cp.tile([P, G], F32, name="gfull")
            nc.vector.memset(gfull[:], 0.0)
            for j in range(NCORES):
                wj = min(GW, G - plan.g_bases[j])
                tw = sb.tile([P, GW], F32, name="twj", tag="ohg")
                nc.sync.dma_start(out=tw[:], in_=gwin_all[j * P:(j + 1) * P, :])
                nc.vector.tensor_add(
                    out=gfull[:, plan.g_bases[j]:plan.g_bases[j] + wj],
                    in0=gfull[:, plan.g_bases[j]:plan.g_bases[j] + wj],
                    in1=tw[:, :wj])

            # ---------------- FFN (replicated on all cores) ----------------
            NGC = math.ceil(G / 512)
            z2sb = cp.tile([P, G], F32, name="z2sb")
            nc.vector.memset(z2sb[:], 0.0)
            for f in range(4):
                z1f = sb.tile([P, G], F32, name="z1f", tag="z1f", bufs=2)
                for gc in range(NGC):
                    g0, g1 = gc * 512, min((gc + 1) * 512, G)
                    pz = ps.tile([P, 512], F32, name="pz", tag="pff",
                                 space="PSUM", bufs=1)
                    nc.tensor.matmul(out=pz[:, :g1 - g0],
                                     lhsT=W1T_s[:, f * P:(f + 1) * P],
                                     rhs=gfull[:, g0:g1], start=True,
                                     stop=True)
                    nc.scalar.activation(out=z1f[:, g0:g1], in_=pz[:, :g1 - g0],
                                         func=_relu(), bias=b1r_s[:, f:f + 1])
                for gc in range(NGC):
                    g0, g1 = gc * 512, min((gc + 1) * 512, G)
                    pz2 = ps.tile([P, 512], F32, name="pz2", tag="pff",
                                  space="PSUM", bufs=1)
                    nc.tensor.matmul(out=pz2[:, :g1 - g0], lhsT=W2T_f[f][:],
                                     rhs=z1f[:, g0:g1], start=True, stop=True)
                    nc.vector.tensor_add(out=z2sb[:, g0:g1],
                                         in0=z2sb[:, g0:g1],
                                         in1=pz2[:, :g1 - g0])
            nc.vector.tensor_add(out=z2sb[:], in0=z2sb[:],
                                 in1=b2r_s[:, 0:1].to_broadcast([P, G]))
            orow = sb.tile([1, G], F32, name="orow", tag="z1f", bufs=2)
            for gc in range(NGC):
                g0, g1 = gc * 512, min((gc + 1) * 512, G)
                po = ps.tile([1, 512], F32, name="po", tag="pff",
                             space="PSUM", bufs=1)
                nc.tensor.matmul(out=po[:, :g1 - g0], lhsT=WlastT_s[:],
                                 rhs=z2sb[:, g0:g1], start=True, stop=True)
                nc.vector.tensor_add(
                    out=orow[:, g0:g1], in0=po[:, :g1 - g0],
                    in1=blast_s[0:1, 0:1].to_broadcast([1, g1 - g0]))
            nc.sync.dma_start(out=out_ext[:, :], in_=orow[:])

    nc.compile()
    if split:
        _split_excess_waits(nc)
    return nc


def _split_excess_waits(nc, max_waits=1):
    k = 0
    for f in nc.m.functions:
        for bb in f.blocks:
            new = []
            for ins in bb.instructions:
                si = ins.sync_info
                if si is not None and len(si.on_wait) > max_waits:
                    waits = list(si.on_wait)
                    for w in waits[:-max_waits]:
                        nop = mybir.InstNoOp(name=f"I-waitsplit-{k}",
                                             engine=ins.engine)
                        k += 1
                        nop.sync_info = mybir.SyncInfo(on_wait=[w],
                                                       on_update=[])
                        new.append(nop)
                    si.on_wait = waits[-max_waits:]
                new.append(ins)
            bb.instructions = new
    return k


# ----------------------------------------------------------------------------
# inputs
# ----------------------------------------------------------------------------

def _in_maps(plan, weights):
    H = plan.H
    GW = plan.GW
    com = {
        "WmT": np.ascontiguousarray(weights["W_m"].T),
        "WieXT": np.ascontiguousarray(weights["W_i"].T),
        "WaxT": np.ascontiguousarray(weights["W_a"][:, :4].T),
        "WahT": np.ascontiguousarray(weights["W_a"][:, 4:].T),
        "W1T": np.ascontiguousarray(weights["W1"].T),
        "W2T": np.ascontiguousarray(weights["W2"].T),
        "WlastT": np.ascontiguousarray(weights["W_last"].T),
        "b1r": np.ascontiguousarray(weights["b1"].reshape(4, H).T),
        "b2r": weights["b2"].reshape(H, 1).copy(),
        "blast": weights["b_last"].reshape(1, 1).copy(),
        "iotaW": np.tile(np.arange(MAXSEG * P, dtype=np.float16), (P, 1)),
        "iotaG": np.tile(np.arange(GW, dtype=np.float32), (P, 1)),
        "ident": np.eye(P, dtype=np.float32),
    }
    maps = []
    for k in range(NCORES):
        info = plan.cores[k]
        m = dict(com)
        m["xT"] = info["xT"]
        m["xeai"] = info["xeai"]
        m["xea1"] = info["xea1"]
        m["batchloc"] = info["batchloc"]
        m["gaL"] = np.concatenate(
            [_wrap_idx16(info["subL"][q]["idx"]) for q in range(4)], axis=1)
        m["gaF"] = np.concatenate(
            [_wrap_idx16(info["subF"][q]["idx"]) for q in range(4)], axis=1)
        m["dlocL"] = np.ascontiguousarray(np.concatenate(
            [info["subL"][q]["dlocf"] for q in range(4)], axis=1))
        m["dlocF"] = np.ascontiguousarray(np.concatenate(
            [info["subF"][q]["dlocf"] for q in range(4)], axis=1))
        m["dloc1"] = np.ascontiguousarray(info["sub1"]["dlocf"])
        maps.append(m)
    return maps


def _prep_all(x, edge_index, edge_attr, batch, depth, weights, G):
    plan = _host_prep(np.asarray(x, np.float32), np.asarray(edge_index),
                      np.asarray(edge_attr, np.float32), np.asarray(batch),
                      int(depth), G)
    maps = _in_maps(plan, weights)
    return plan, maps


def kernel(x, edge_index, edge_attr, batch, depth,
           W_i, W_m, W_a, W1, b1, W2, b2, W_last, b_last):
    weights = {
        "W_i": np.asarray(W_i, np.float32), "W_m": np.asarray(W_m, np.float32),
        "W_a": np.asarray(W_a, np.float32), "W1": np.asarray(W1, np.float32),
        "b1": np.asarray(b1, np.float32), "W2": np.asarray(W2, np.float32),
        "b2": np.asarray(b2, np.float32),
        "W_last": np.asarray(W_last, np.float32),
        "b_last": np.asarray(b_last, np.float32),
    }
    G = 2048
    plan, maps = _prep_all(x, edge_index, edge_attr, batch, depth, weights, G)
    nc = _build(plan, split=True)
    res = run_bass_kernel_spmd(nc, maps, list(range(NCORES)))
    return np.asarray(res.results[0]["out"]).reshape(G, 1).astype(np.float32)
